# revision 9
# baseline (speedup 1.0000x reference)
"""Trainium2 Bass kernel for CosineAttention:

    out = sigmoid((xn @ xn.T) @ x)   where xn = x / ||x_row||

Key algebraic optimization: reassociate (xn @ xn.T) @ x = xn @ (xn.T @ x).
G = xn.T @ x is [D, D] - the O(N^2 D) similarity matrix is never formed.

Sharding: rows of x across 8 cores. Each core:
  1. loads its [N/8, D] row block, computes row norms + normalized rows
  2. computes partial G'_c = xn_c.T @ x_c - (c/8)*I  (f32 PSUM accum)
  3. AllReduce across the 8 cores (fp16 payload)
  4. out_c = sigmoid(xn_c @ G' + c*xn_c)
The host concatenates the 8 row blocks.

G is symmetric, so only the left column-half (cols 0:512, 1MB) and the
lower-right quadrant (rows/cols 512:1024, 512KB) are AllReduced; the
upper-right quadrant is reconstructed on-chip by PE-transposing the
lower-left blocks of the first AllReduce result.

The c*xn correction is applied on the PE as one extra accumulating
matmul per output tile: psz += (c*I).T @ xnr_rowblock. This keeps the
AllReduce payload free of the large diagonal (fp16-roundable) without
any DVE adds or staged c*xn tiles.

Observed TOPSP behavior: the first collective mesh begins only ~10us
after the LAST doorbell in the NEFF has fired, and meshes execute
serially. So both doorbells are pushed as early as the data
dependencies allow, and no warmup collective is used.
"""

import numpy as np

import concourse.bass as bass  # noqa: F401
import concourse.mybir as mybir
import concourse.tile as tile
from concourse import bacc
from concourse.bass_utils import run_bass_kernel_spmd
from concourse.masks import make_identity

F32 = mybir.dt.float32
BF16 = mybir.dt.bfloat16
F16 = mybir.dt.float16
AFT = mybir.ActivationFunctionType

N, D = 8192, 1024
NCORES = 8
R = N // NCORES  # rows per core
P = 128
RT = R // P      # row tiles per core
KT = D // P      # contraction tiles (mm2) / G row tiles
FD = 512         # matmul moving free dim (one PSUM bank of f32)
NH = D // FD     # column halves
QT = KT // 2     # tiles per half (4)
GROUPS = [list(range(NCORES))]
DIAG_C = 256.0   # ~mean of diag(G); exact in fp16


def _emit_body(tc, xb, out, ctx):
    nc = tc.nc
    mm_dt = F16
    xb_t = xb.rearrange("(rt p) d -> rt p d", p=P)
    out_t = out.rearrange("(rt p) d -> rt p d", p=P)

    persist = ctx.enter_context(tc.tile_pool(name="persist", bufs=1))
    load = ctx.enter_context(tc.tile_pool(name="load", bufs=3))
    small = ctx.enter_context(tc.tile_pool(name="small", bufs=1))
    ostage = ctx.enter_context(tc.tile_pool(name="ostage", bufs=8))
    ps = ctx.enter_context(tc.tile_pool(name="ps", bufs=1, space="PSUM"))
    dram = ctx.enter_context(tc.tile_pool(name="dram", bufs=1, space="DRAM"))

    # ---- warmup collective: absorbs first-collective ncfw setup (the
    # first mesh otherwise pays ~10us extra). The TOPSP begins its
    # first mesh only after the LAST doorbell in the NEFF fires, so
    # this just prepends a cheap 8us mesh to the train while removing
    # AR1's setup penalty.
    w_in = dram.tile([P, 4], F32, tag="w_in")
    w_out = dram.tile([P * NCORES, 4], F32, tag="w_out", addr_space="Shared")
    nc.gpsimd.collective_compute(
        "AllGather", mybir.AluOpType.bypass, replica_groups=GROUPS,
        ins=[w_in.opt()], outs=[w_out.opt()],
    )

    # ---- phase 0: chunked loads, cast to fp16, norms ----
    # The row block is loaded in 4 chunks of 2 row-tiles (1MB each) so
    # completions stagger instead of bunching at the 11.7us BW floor.
    # ACT Squares come straight off the f32 halves with free-axis
    # accumulate; three sqrt batches track the chunk arrivals. xn and
    # the fp16 cast run on DVE.
    CH = 2  # row tiles per load chunk
    xfall = persist.tile([P, RT, D], F32, tag="xfall")
    xb_c = xb.rearrange("(c q p) d -> c p q d", p=P, q=CH)
    for c in range(RT // CH):
        nc.sync.dma_start(out=xfall[:, c * CH:(c + 1) * CH, :], in_=xb_c[c])
    xbr, xnr = [], []
    ssA = small.tile([P, RT], F32, tag="ssA")
    ssB = small.tile([P, RT], F32, tag="ssB")
    ss_all = small.tile([P, RT], F32, tag="ss_all")
    nrm_all = small.tile([P, RT], F32, tag="nrm_all")
    rn_all = small.tile([P, RT], F32, tag="rn_all")
    BATCH_END = {1: (0, 2), 3: (2, 4), 7: (4, 8)}
    for rt in range(RT):
        xf = xfall[:, rt, :]
        sqa = load.tile([P, FD], BF16, tag="sqa")
        nc.scalar.activation(out=sqa, in_=xf[:, :FD], func=AFT.Square,
                             accum_out=ssA[:, rt:rt + 1])
        sqb = load.tile([P, FD], BF16, tag="sqb")
        nc.scalar.activation(out=sqb, in_=xf[:, FD:], func=AFT.Square,
                             accum_out=ssB[:, rt:rt + 1])
        t_xbr = persist.tile([P, D], mm_dt, tag=f"xbr{rt}", name=f"xbr{rt}")
        nc.vector.tensor_copy(out=t_xbr, in_=xf)
        xbr.append(t_xbr)
        if rt in BATCH_END:
            lo, hi = BATCH_END[rt]
            nc.vector.tensor_add(ss_all[:, lo:hi], ssA[:, lo:hi],
                                 ssB[:, lo:hi])
            nc.scalar.sqrt(nrm_all[:, lo:hi], ss_all[:, lo:hi])
            nc.vector.reciprocal(rn_all[:, lo:hi], nrm_all[:, lo:hi])
            for rr in range(lo, hi):
                t_xnr = persist.tile([P, D], mm_dt, tag=f"xnr{rr}",
                                     name=f"xnr{rr}")
                nc.vector.tensor_scalar_mul(t_xnr, xbr[rr],
                                            rn_all[:, rr:rr + 1])
                xnr.append(t_xnr)

    # identity / diag-shift constants (emitted after the loads so their
    # DVE/ACT setup doesn't delay the load-issue critical path)
    identb = persist.tile([P, P], mm_dt, tag="identb")
    make_identity(nc, identb)
    identc = persist.tile([P, P], mm_dt, tag="identc")
    nc.scalar.mul(identc, identb, DIAG_C)
    dsh = []
    for s in range(FD // P):
        t_dsh = persist.tile([P, FD], mm_dt, tag=f"dsh{s}", name=f"dsh{s}")
        nc.vector.memset(t_dsh, 0.0)
        nc.scalar.mul(t_dsh[:, s * P:(s + 1) * P], identb, -DIAG_C / NCORES)
        dsh.append(t_dsh)

    # ---- phase 1a: G' cols 0:512 = xn_c.T @ x_c[:, 0:512] (- c/8*I) ----
    g_in0 = dram.tile([D, FD], mm_dt, tag="g_in0")
    g_out0 = dram.tile([D, FD], mm_dt, tag="g_out0", addr_space="Shared")
    g_in0_g = g_in0.rearrange("(g q p) f -> g p q f", p=P, q=QT)
    g_out0_g = g_out0.rearrange("(g q p) f -> g p q f", p=P, q=QT)

    psg0 = [ps.tile([P, FD], F32, tag=f"acc{mt}", name=f"psg0_{mt}")
            for mt in range(KT)]
    for rt in range(RT):
        for mt in range(KT):
            nc.tensor.matmul(
                psg0[mt],
                lhsT=xnr[rt][:, mt * P:(mt + 1) * P],
                rhs=xbr[rt][:, 0:FD],
                start=(rt == 0),
                stop=(rt == RT - 1) and mt >= QT,
            )
    for mt in range(QT):
        # diag blocks live at mt 0..3 for the left column-half
        nc.tensor.matmul(psg0[mt], lhsT=identb, rhs=dsh[mt],
                         start=False, stop=True)

    gA = [persist.tile([P, QT, FD], mm_dt, tag=f"gA{g}", name=f"gA{g}")
          for g in range(2)]
    for mt in range(KT):
        g, q = divmod(mt, QT)
        if mt < QT:
            nc.vector.tensor_copy(out=gA[g][:, q, :], in_=psg0[mt])
        else:
            nc.scalar.copy(out=gA[g][:, q, :], in_=psg0[mt])
    nc.sync.dma_start(out=g_in0_g[0], in_=gA[0])
    nc.scalar.dma_start(out=g_in0_g[1], in_=gA[1])
    nc.gpsimd.collective_compute(
        "AllReduce", mybir.AluOpType.add, replica_groups=GROUPS,
        ins=[g_in0.opt()], outs=[g_out0.opt()],
    )

    # ---- phase 1b: G' lower-right quadrant rows/cols 512:1024 ----
    g_in1 = dram.tile([FD, FD], mm_dt, tag="g_in1")
    g_out1 = dram.tile([FD, FD], mm_dt, tag="g_out1", addr_space="Shared")
    g_in1_g = g_in1.rearrange("(q p) f -> p q f", p=P)
    g_out1_g = g_out1.rearrange("(q p) f -> p q f", p=P)

    psg1 = [ps.tile([P, FD], F32, tag=f"acc{QT + q}", name=f"psg1_{q}")
            for q in range(QT)]
    for rt in range(RT):
        for q in range(QT):
            nc.tensor.matmul(
                psg1[q],
                lhsT=xnr[rt][:, (QT + q) * P:(QT + q + 1) * P],
                rhs=xbr[rt][:, FD:],
                start=(rt == 0),
                stop=False,
            )
    for q in range(QT):
        nc.tensor.matmul(psg1[q], lhsT=identb, rhs=dsh[q],
                         start=False, stop=True)
    gB = persist.tile([P, QT, FD], mm_dt, tag="gB")
    for q in range(QT):
        if q % 2 == 0:
            nc.vector.tensor_copy(out=gB[:, q, :], in_=psg1[q])
        else:
            nc.scalar.copy(out=gB[:, q, :], in_=psg1[q])
    nc.sync.dma_start(out=g_in1_g, in_=gB)
    nc.gpsimd.collective_compute(
        "AllReduce", mybir.AluOpType.add, replica_groups=GROUPS,
        ins=[g_in1.opt()], outs=[g_out1.opt()],
    )

    # ---- phase 1c (hidden in AR windows): xnT transposes ----
    xnT = []
    for kt in range(KT):
        t_xnT = persist.tile([P, D], mm_dt, tag=f"xnT{kt}", name=f"xnT{kt}")
        for rt in range(RT):
            src = xnr[rt][:, kt * P:(kt + 1) * P]
            tpt = ps.tile([P, P], mm_dt, tag=f"acc{rt % 2}",
                          name=f"tp{kt}_{rt}")
            nc.tensor.transpose(tpt, src, identb)
            if rt % 2 == 0:
                nc.vector.tensor_copy(out=t_xnT[:, rt * P:(rt + 1) * P],
                                      in_=tpt)
            else:
                nc.scalar.copy(out=t_xnT[:, rt * P:(rt + 1) * P], in_=tpt)
        xnT.append(t_xnT)

    # ---- phase 2: load G chunk 0, reconstruct upper-right, mm2 ----
    grh = [persist.tile([P, QT, FD], mm_dt, tag=f"grh{g}", name=f"grh{g}")
           for g in range(2)]
    nc.sync.dma_start(out=grh[0], in_=g_out0_g[0])
    nc.scalar.dma_start(out=grh[1], in_=g_out0_g[1])
    gr0 = [grh[kt // QT][:, kt % QT, :] for kt in range(KT)]

    # upper-right quadrant G'[0:512, 512:1024] = blockwise transpose of
    # G'[512:1024, 0:512] (= gr0[4..7])
    grT = [persist.tile([P, FD], mm_dt, tag=f"grT{q}", name=f"grT{q}")
           for q in range(QT)]
    for q in range(QT):          # target row-block q (cols 512:1024)
        for b in range(QT):      # source row-block 4+b
            tpq = ps.tile([P, P], mm_dt, tag=f"acc{b % 2}",
                          name=f"tpq{q}_{b}")
            nc.tensor.transpose(tpq, gr0[QT + b][:, q * P:(q + 1) * P],
                                identb)
            if b % 2 == 0:
                nc.vector.tensor_copy(out=grT[q][:, b * P:(b + 1) * P],
                                      in_=tpq)
            else:
                nc.scalar.copy(out=grT[q][:, b * P:(b + 1) * P], in_=tpq)

    def mm2_half(nh, gr):
        psz = [ps.tile([P, FD], F32, tag=f"acc{mt}", name=f"psz{nh}_{mt}")
               for mt in range(RT)]
        for kt in range(KT):
            for mt in range(RT):
                nc.tensor.matmul(
                    psz[mt],
                    lhsT=xnT[kt][:, mt * P:(mt + 1) * P],
                    rhs=gr[kt],
                    start=(kt == 0),
                    stop=False,
                )
        for mt in range(RT):
            # += c * xn rowblock: (c*I).T @ xnr[mt] -- replaces the DVE
            # cxn add and closes the accumulation group
            nc.tensor.matmul(
                psz[mt], lhsT=identc,
                rhs=xnr[mt][:, nh * FD:(nh + 1) * FD],
                start=False, stop=True,
            )
            ob = ostage.tile([P, FD], F32, tag="ob")
            nc.scalar.activation(out=ob, in_=psz[mt], func=AFT.Sigmoid)
            lo = nh * FD
            if nh == 1 and mt >= RT - 2:
                # tail stores: split across both rings so the last store
                # isn't a single 256KB transfer on the critical path
                HB = FD // 2
                nc.sync.dma_start(out=out_t[mt][:, lo:lo + HB],
                                  in_=ob[:, :HB])
                nc.scalar.dma_start(out=out_t[mt][:, lo + HB:lo + FD],
                                    in_=ob[:, HB:])
            else:
                eng = nc.sync if mt % 2 == 0 else nc.scalar
                eng.dma_start(out=out_t[mt][:, lo:lo + FD], in_=ob)

    mm2_half(0, gr0)

    # ---- phase 3: after AR2, assemble col-half 1 rhs and finish ----
    grq_t = persist.tile([P, QT, FD], mm_dt, tag="grq_t")
    nc.sync.dma_start(out=grq_t, in_=g_out1_g)
    grq = [grq_t[:, q, :] for q in range(QT)]
    mm2_half(1, grT + grq)


def build():
    from contextlib import ExitStack

    nc = bacc.Bacc("TRN2", target_bir_lowering=False, debug=False,
                   num_devices=NCORES)
    xb = nc.dram_tensor("xb", [R, D], F32, kind="ExternalInput").ap()
    out = nc.dram_tensor("out", [R, D], F32, kind="ExternalOutput").ap()
    with tile.TileContext(nc) as tc:
        with ExitStack() as ctx:
            _emit_body(tc, xb, out, ctx)
    nc.compile()
    return nc


_NC_CACHE = {}


def _get_nc():
    if "nc" not in _NC_CACHE:
        _NC_CACHE["nc"] = build()
    return _NC_CACHE["nc"]


def kernel(x: np.ndarray) -> np.ndarray:
    x = np.asarray(x, dtype=np.float32)
    assert x.shape == (N, D), x.shape
    nc = _get_nc()
    in_maps = [{"xb": x[c * R:(c + 1) * R]} for c in range(NCORES)]
    res = run_bass_kernel_spmd(nc, in_maps, list(range(NCORES)))
    return np.concatenate([res.results[c]["out"] for c in range(NCORES)], axis=0)
